# revision 1
# baseline (speedup 1.0000x reference)
"""Trainium2 Bass kernel for nn_CrossAttention (gnn_message_passing).

Reference computation (per batch b, point n):
  nb[c,n,o]  = sum_f neighbors[c,n,f] * W_two[o,f] + b_two[o]
  q[n,e]     = sum_c pcd[n,c] Wq[e,c]
  k[e,n,o]   = sum_c Wk[e,c] nb[c,n,o]
  v[e,n,o]   = sum_c Wv[e,c] nb[c,n,o]
  scores     = sum_d q[n,(h,d)] k[(h,d),n,o] / sqrt(8)
  attn       = softmax_o(scores)
  out[(h,d),n] = sum_o attn[h,n,o] v[(h,d),n,o]

Sharding: data-parallel over (b, n-block): 8 cores, each takes 256 points.

Device pipeline per core (n=256 points, c=64, f=512, o=256, h=8, d=8):
  S1: nb[(n,c), o] via fp32r matmuls, lhsT = host-transposed neighbors chunks
      [f=128, cn=128] (stationary), rhs = W_two^T chunks [f=128, o=256].
  S2: per (n, o-half): out[o-half=128, 128] = nb_n[c=64, o-half]^T @
      [Wv^T | qc_{8n-block}] -> v_T (cols 0-63) + scores_T (col 64+8j+h).
      qc[h,n,c] = sum_d q[n,(h,d)] Wk[(h,d),c]/sqrt(8) precomputed on host
      (19 MFLOP, 0.4% of total; pure reparametrization of q@k).
  softmax: scores stay [o-partitions, free]; exp on ACT (no max-subtract:
      |scores| ~ 0.05 for this problem's scales); Z via ones-matmul.
  S4: xc[0:64, h] = sum_o v_T[o,e] exp[o,h], xc[64:72, h] = Z[h] via
      ones-columns packed into the v tile. Normalize at the end.
"""

import math
import ml_dtypes
import numpy as np
from contextlib import ExitStack

import concourse.bass as bass
import concourse.tile as tile
from concourse import bacc, mybir
from concourse.bass_utils import run_bass_kernel_spmd

F32R = mybir.dt.float32r
F32 = mybir.dt.float32
BF16 = mybir.dt.bfloat16

NCORES = 8
B, N, C, LF = 2, 1024, 64, 256
F2 = 2 * LF          # 512 neighbor features
O = LF               # 256 attention keys per point
H, D = 8, 8          # heads, depth
NP = (B * N) // NCORES  # 256 points per core
G = NP // 8          # 32 groups of 8 points

_BUILD_CACHE = {}
STAGE = 4  # debug: 1=S1 only, 2=+S2, 3=+exp, 4=full
FEATURES = {"s2mm", "vevac", "stevac", "memset", "exp", "s4"}


def build_nc(with_bias: bool, repeat: int = 1, g_mod: int = G):
    """Build the per-core Bass module.

    g_mod: number of groups present in the nbt input (the g-loop reads
    nbt[g % g_mod]); g_mod == G for real runs, smaller for timing builds.
    repeat: device-side repetition count (For_i) for timing.
    """
    key = (with_bias, repeat, g_mod, STAGE, tuple(sorted(FEATURES)))
    if key in _BUILD_CACHE:
        return _BUILD_CACHE[key]

    nc = bacc.Bacc("TRN2", target_bir_lowering=False, debug=False)
    # DRAM I/O
    nbt_d = nc.dram_tensor("nbt", [g_mod, 4, 128, 512], F32R, kind="ExternalInput").ap()
    w2t_d = nc.dram_tensor("w2t", [4, 128, 256], F32R, kind="ExternalInput").ap()
    r2_d = nc.dram_tensor("r2", [G, 128, 128], BF16, kind="ExternalInput").ap()
    b2_d = nc.dram_tensor("b2", [1, 256], F32R, kind="ExternalInput").ap()
    xcout_d = nc.dram_tensor("xcout", [72, NP, 8], F32, kind="ExternalOutput").ap()

    with tile.TileContext(nc) as tc, ExitStack() as ctx:
        singles = ctx.enter_context(tc.tile_pool(name="singles", bufs=1))
        slabs = ctx.enter_context(tc.tile_pool(name="slabs", bufs=3))
        nbs = ctx.enter_context(tc.tile_pool(name="nbs", bufs=3))
        vs = ctx.enter_context(tc.tile_pool(name="vs", bufs=3))
        sts = ctx.enter_context(tc.tile_pool(name="sts", bufs=3))
        ps_nb = ctx.enter_context(tc.tile_pool(name="ps_nb", bufs=1, space="PSUM"))
        ps_vq = ctx.enter_context(tc.tile_pool(name="ps_vq", bufs=2, space="PSUM"))
        ps_xc = ctx.enter_context(tc.tile_pool(name="ps_xc", bufs=2, space="PSUM"))

        # one-time loads
        w2t = singles.tile([128, 4, 256], F32R)
        nc.sync.dma_start(out=w2t, in_=w2t_d.rearrange("a p c -> p a c"))
        r2 = singles.tile([128, G, 128], BF16)
        nc.sync.dma_start(out=r2, in_=r2_d.rearrange("g p c -> p g c"))
        if with_bias:
            b2 = singles.tile([1, 256], F32R)
            nc.sync.dma_start(out=b2, in_=b2_d)
            ones1 = singles.tile([1, 128], F32R)
            nc.vector.memset(ones1.bitcast(F32), 1.0)

        xc_pool = ctx.enter_context(tc.tile_pool(name="xc_full", bufs=1))
        xc_holder = {}

        def body(_i=None):
            xc_full = xc_pool.tile([128, NP, 8], F32, tag="xcf")
            xc_holder["t"] = xc_full
            nc.gpsimd.memset(xc_full, 0.0)
            for g in range(G):
                gi = g % g_mod
                # ---- S1: nb[(n,c), o] for the 8 points of this group ----
                slab = slabs.tile([128, 4, 512], F32R, tag="slab")
                nc.gpsimd.dma_start(out=slab, in_=nbt_d[gi].rearrange("a p c -> p a c"))
                nb_ps = ps_nb.tile([128, 1024], F32, tag="nbps")
                for t in range(4):
                    for ci in range(4):
                        nc.tensor.matmul(
                            nb_ps[:, 256 * t : 256 * t + 256],
                            slab[:, ci, 128 * t : 128 * t + 128],
                            w2t[:, ci, :],
                            start=(ci == 0),
                            stop=(ci == 3) and not with_bias,
                        )
                    if with_bias:
                        nc.tensor.matmul(
                            nb_ps[:, 256 * t : 256 * t + 256],
                            ones1,
                            b2,
                            start=False,
                            stop=True,
                        )
                nb_sb = nbs.tile([128, 4, 256], BF16, tag="nb")
                nc.vector.tensor_copy(nb_sb[:, 0:2, :], nb_ps[:, 0:512])
                nc.scalar.copy(nb_sb[:, 2:4, :], nb_ps[:, 512:1024])
                # odd-n copies shifted to partition base 0 (HW rejects K=64
                # matmuls with operands at partition base 64)
                nb_od = nbs.tile([64, 4, 256], BF16, tag="nbod")
                nc.vector.tensor_copy(nb_od[:, 0:2, :], nb_ps[64:128, 0:512])
                nc.scalar.copy(nb_od[:, 2:4, :], nb_ps[64:128, 512:1024])

                # ---- S2: v_T + scores_T per (n, o-half) ----
                if STAGE < 2:
                    continue
                exp_sb = sts.tile([128, 16, 8], BF16, tag="exp")
                v_g = vs.tile([128, 16, 128], BF16, tag="v")
                # ones cols 64-72, zeros 72-128 for the S4 stationary tiles
                if "memset" in FEATURES:
                    nc.gpsimd.memset(v_g[:, :, 64:72], 1.0)
                    nc.gpsimd.memset(v_g[:, :, 72:128], 0.0)
                for a in range(2):
                    vq = ps_vq.tile([128, 1024], F32, tag="vq")
                    for m in range(4):
                        nl = 4 * a + m       # n within group (0..7)
                        t = nl // 2          # nb subtile
                        par = nl % 2
                        src = nb_sb if par == 0 else nb_od
                        for half in range(2):
                            nc.tensor.matmul(
                                vq[:, 128 * (2 * m + half) : 128 * (2 * m + half) + 128],
                                src[0:64, t, 128 * half : 128 * half + 128],
                                r2[0:64, g, :],
                                start=True,
                                stop=True,
                            )
                    # v_T evac (cols 0..64 of each slot) on ACT, cast bf16
                    vq3 = vq.rearrange("p (s x) -> p s x", s=8)
                    if "vevac" in FEATURES:
                        nc.scalar.copy(v_g[:, 8 * a : 8 * a + 8, 0:64], vq3[:, :, 0:64])
                    # scores_T evac: col 64+8*(4a+m)+h of slot (2m+half)
                    st_in = bass.AP(
                        tensor=vq.tensor,
                        offset=vq.offset + 64 + 32 * a,
                        ap=[vq.ap[0], [264, 4], [128, 2], [1, 8]],
                    )
                    st_sb = sts.tile([128, 2, 4, 2, 8], F32, tag="st")
                    if "stevac" in FEATURES:
                        nc.vector.tensor_copy(st_sb[:, a], st_in)
                    # exp on ACT -> bf16 (no max subtraction; |scores| << 1)
                    if STAGE < 3:
                        continue
                    nc.scalar.activation(
                        out=exp_sb[:, 8 * a : 8 * a + 8, :].rearrange("p s x -> p (s x)"),
                        in_=st_sb[:, a].rearrange("p a b c -> p (a b c)"),
                        func=mybir.ActivationFunctionType.Exp,
                        scale=1.0,
                    )

                # ---- S4: xc[e|Z, h] per n, accumulate o-halves ----
                if STAGE < 4:
                    continue
                xc_ps = ps_xc.tile([128, 64], F32, tag="xc")
                for nl in range(8):
                    a, m = nl // 4, nl % 4
                    for half in range(2):
                        slot = 8 * a + 2 * m + half
                        nc.tensor.matmul(
                            xc_ps[:, 8 * nl : 8 * nl + 8],
                            v_g[:, slot, :],
                            exp_sb[:, slot, :],
                            start=(half == 0),
                            stop=(half == 1),
                        )
                nc.vector.tensor_copy(xc_full[:, 8 * g : 8 * g + 8, :], xc_ps)

        if repeat > 1:
            with tc.For_i(0, repeat, 1):
                body()
        else:
            body()

        # ---- tail: ship raw xc (x rows 0-63, Z replicas rows 64-71) ----
        xc_full = xc_holder["t"]
        nc.sync.dma_start(out=xcout_d, in_=xc_full[0:72])

    nc.compile()
    _BUILD_CACHE[key] = nc
    return nc


def host_prep(pcd, neighbors, W_two, b_two, Wq, Wk, Wv):
    """Per-core input maps (host-side layout transforms + q/qc fold)."""
    scale = 1.0 / math.sqrt(D)
    # q[b,n,e] then qc[b,h,n,c] = sum_d q[b,n,(h,d)] Wk[(h,d),c] * scale
    q = np.einsum("bnc,ec->bne", pcd, Wq).astype(np.float32)
    qc = np.einsum("bnhd,hdc->bhnc", q.reshape(B, N, H, D), Wk.reshape(H, D, C))
    qc = (qc * scale).astype(np.float32)

    w2t = np.ascontiguousarray(W_two.T.reshape(4, 128, O)).astype(np.float32)
    b2 = b_two.reshape(1, O).astype(np.float32)
    with_bias = bool(np.any(b_two))

    in_maps = []
    npb = N // (NCORES // B)  # points per core
    for core in range(NCORES):
        b = core // (NCORES // B)
        n0 = (core % (NCORES // B)) * npb
        nb = neighbors[b, :, n0 : n0 + npb, :]          # (c, np, f)
        # nbt[g, ci, fi, cn] with cn = (n within group)*64 + c
        nbt = np.transpose(nb, (2, 1, 0)).reshape(F2, G, 8 * C)   # (f, g, cn)
        nbt = np.transpose(nbt, (1, 0, 2)).reshape(G, 4, 128, 8 * C)
        nbt = np.ascontiguousarray(nbt).astype(np.float32)
        # r2[g, c(x2), col]: cols 0-63 = Wv^T, 64+8j+h = qc[h, 8g+j, c]
        r2 = np.zeros((G, 128, 128), np.float32)
        r2[:, 0:64, 0:64] = np.broadcast_to(Wv.T, (G, C, C))
        qc_core = qc[b, :, n0 : n0 + npb, :]             # (h, np, c)
        # [g, c, 8j+h]
        qjc = np.transpose(qc_core.reshape(H, G, 8, C), (1, 3, 2, 0)).reshape(G, C, 64)
        r2[:, 0:64, 64:128] = qjc
        r2[:, 64:128, :] = r2[:, 0:64, :]
        r2 = r2.astype(ml_dtypes.bfloat16)
        in_maps.append({"nbt": nbt, "w2t": w2t, "r2": r2, "b2": b2})
    return in_maps, with_bias


def kernel(pcd, neighbors, W_two, b_two, Wq, Wk, Wv):
    in_maps, with_bias = host_prep(pcd, neighbors, W_two, b_two, Wq, Wk, Wv)
    nc = build_nc(with_bias)
    res = run_bass_kernel_spmd(nc, in_maps, list(range(NCORES)))
    out = np.empty((B, C, N), np.float32)
    npb = N // (NCORES // B)
    hh = np.arange(C) // D  # head index per output channel
    for core in range(NCORES):
        b = core // (NCORES // B)
        n0 = (core % (NCORES // B)) * npb
        xc = res.results[core]["xcout"]          # [72, NP, 8]
        x = xc[np.arange(C), :, hh]              # [C, NP] numerator
        z = xc[64, :, hh]                        # [C, NP] denominator (Z replicas)
        out[b, :, n0 : n0 + npb] = x / z
    return out



# revision 7
# speedup vs baseline: 2.0719x; 2.0719x over previous
"""Trainium2 Bass kernel for nn_CrossAttention (gnn_message_passing).

Reference computation (per batch b, point n):
  nb[c,n,o]  = sum_f neighbors[c,n,f] * W_two[o,f] + b_two[o]
  q[n,e]     = sum_c pcd[n,c] Wq[e,c]
  scores     = sum_c qc[h,n,c] nb[c,n,o]   (qc = q@Wk reparametrized, /sqrt(8))
  attn       = softmax_o(scores)
  out[(h,d),n] = sum_o attn[h,n,o] v[(h,d),n,o],  v = Wv @ nb

Sharding: data-parallel over (b, n-block): 8 cores, each takes 256 points.

Device pipeline per core (n=256 points, c=64, f=512, o=256, h=8, d=8),
all matmuls bf16 (inputs host-cast), G=32 groups of 8 points:
  S1: nb[(n,c), o] per group: 16 matmuls [f=128,cn=128]^T @ [f=128,o=256]
      accumulated over 4 f-chunks -> PSUM [cn=128 x 4 slices, o=256].
  S2: per point-PAIR (cn=128 = 2 points x 64 ch) and o-half: one matmul
      stationary nb[cn=128, o=128], rhs = [blockdiag(Wv^T,Wv^T) | qc_pair]
      (144 cols) -> PSUM [o=128, 144]: v_T for both points + scores_T.
  softmax: exp on ACT straight from PSUM score cols (no max-subtract:
      |scores| ~ 0.05 at this problem's scales), bf16 out.
  S4: stationary = exp[o=128, h=8] (8-col weight load), rhs = v_T cols +
      shared ones column (gives Z) -> PSUM [8, 65] per point at partition
      base 32*(nl%4). Normalize x/Z on host.
Emission is software-pipelined S2(g) -> S1(g+1) -> S4(g) so evacuations
(DVE/ACT) hide under tensor work of the neighboring stage.
"""

import math
import ml_dtypes
import numpy as np
from contextlib import ExitStack

import concourse.bass as bass
import concourse.tile as tile
from concourse import bacc, mybir
from concourse.bass_utils import run_bass_kernel_spmd

F32 = mybir.dt.float32
BF16 = mybir.dt.bfloat16

NCORES = 8
B, N, C, LF = 2, 1024, 64, 256
F2 = 2 * LF          # 512 neighbor features
O = LF               # 256 attention keys per point
H, D = 8, 8          # heads, depth
NP = (B * N) // NCORES  # 256 points per core
G = NP // 8          # 32 groups of 8 points

_BUILD_CACHE = {}


def build_nc(with_bias: bool, repeat: int = 1, g_mod: int = G):
    """Build the per-core Bass module.

    g_mod: number of groups present in nbt/s2r inputs (the g-loop reads
    index g % g_mod); g_mod == G for real runs, smaller for timing builds.
    repeat: device-side repetition count (For_i) for timing.
    """
    key = (with_bias, repeat, g_mod)
    if key in _BUILD_CACHE:
        return _BUILD_CACHE[key]

    nc = bacc.Bacc("TRN2", target_bir_lowering=False, debug=False)
    nbt_d = nc.dram_tensor("nbt", [g_mod, 128, 4, 512], BF16, kind="ExternalInput").ap()
    w2t_d = nc.dram_tensor("w2t", [128, 4, 256], BF16, kind="ExternalInput").ap()
    s2r_d = nc.dram_tensor("s2r", [g_mod, 128, 4, 144], BF16, kind="ExternalInput").ap()
    b2_d = nc.dram_tensor("b2", [1, 256], BF16, kind="ExternalInput").ap()
    xcout_d = nc.dram_tensor("xcout", [G, 8, 8, 66], F32, kind="ExternalOutput").ap()

    with tile.TileContext(nc) as tc, ExitStack() as ctx:
        singles = ctx.enter_context(tc.tile_pool(name="singles", bufs=1))
        slabs = ctx.enter_context(tc.tile_pool(name="slabs", bufs=3))
        s2rs = ctx.enter_context(tc.tile_pool(name="s2rs", bufs=3))
        nbs = ctx.enter_context(tc.tile_pool(name="nbs", bufs=2))
        vs = ctx.enter_context(tc.tile_pool(name="vs", bufs=2))
        es = ctx.enter_context(tc.tile_pool(name="es", bufs=2))
        xs = ctx.enter_context(tc.tile_pool(name="xs", bufs=3))
        ps_nb = ctx.enter_context(tc.tile_pool(name="ps_nb", bufs=1, space="PSUM"))
        ps_vq = ctx.enter_context(tc.tile_pool(name="ps_vq", bufs=4, space="PSUM"))
        ps_xca = ctx.enter_context(tc.tile_pool(name="ps_xca", bufs=1, space="PSUM"))
        ps_xcb = ctx.enter_context(tc.tile_pool(name="ps_xcb", bufs=1, space="PSUM"))

        w2t = singles.tile([128, 4, 256], BF16)
        nc.sync.dma_start(out=w2t, in_=w2t_d)
        if with_bias:
            b2 = singles.tile([1, 256], BF16)
            nc.sync.dma_start(out=b2, in_=b2_d)
            ones1 = singles.tile([1, 128], BF16)
            nc.gpsimd.memset(ones1, 1.0)

        def emit_s1(g):
            gi = g % g_mod
            slab = slabs.tile([128, 4, 512], BF16, tag="slab")
            nc.gpsimd.dma_start(out=slab, in_=nbt_d[gi])
            s2r = s2rs.tile([128, 4, 144], BF16, tag="s2r")
            nc.gpsimd.dma_start(out=s2r, in_=s2r_d[gi])
            nb_ps = ps_nb.tile([128, 1024], F32, tag="nbps")
            for t in range(4):
                for ci in range(4):
                    nc.tensor.matmul(
                        nb_ps[:, 256 * t : 256 * t + 256],
                        slab[:, ci, 128 * t : 128 * t + 128],
                        w2t[:, ci, :],
                        start=(ci == 0),
                        stop=(ci == 3) and not with_bias,
                    )
                if with_bias:
                    nc.tensor.matmul(
                        nb_ps[:, 256 * t : 256 * t + 256],
                        ones1, b2, start=False, stop=True,
                    )
            nb_sb = nbs.tile([128, 4, 256], BF16, tag="nb")
            nc.vector.tensor_copy(nb_sb[:, 0:2, :], nb_ps[:, 0:512])
            nc.scalar.copy(nb_sb[:, 2:4, :], nb_ps[:, 512:1024])
            return nb_sb, s2r

        def emit_s2(nb_sb, s2r):
            v_sb = vs.tile([128, 4, 2, 130], BF16, tag="v")
            exp_sb = es.tile([128, 4, 2, 16], BF16, tag="exp")
            nc.gpsimd.memset(v_sb[:, :, :, 64:65], 1.0)
            for t in range(4):
                vq = ps_vq.tile([128, 2, 144], F32, tag="vq")
                for oh in range(2):
                    nc.tensor.matmul(
                        vq[:, oh, :],
                        nb_sb[:, t, 128 * oh : 128 * oh + 128],
                        s2r[:, t, :],
                        start=True,
                        stop=True,
                    )
                nc.vector.tensor_copy(v_sb[:, t, :, 0:64], vq[:, :, 0:64])
                nc.vector.tensor_copy(v_sb[:, t, :, 65:129], vq[:, :, 64:128])
                nc.scalar.activation(
                    out=exp_sb[:, t],
                    in_=vq[:, :, 128:144],
                    func=mybir.ActivationFunctionType.Exp,
                    scale=1.0,
                )
            return v_sb, exp_sb

        def emit_s4(g, v_sb, exp_sb):
            xca = ps_xca.tile([8, 4, 72], F32, tag="xca")
            xcb = ps_xcb.tile([8, 4, 72], F32, tag="xcb")
            for nl in range(8):
                t, par = nl // 2, nl % 2
                xcg = xca if nl < 4 else xcb
                for oh in range(2):
                    nc.tensor.matmul(
                        xcg[0:8, nl % 4, 0:65],
                        exp_sb[:, t, oh, 8 * par : 8 * par + 8],
                        v_sb[:, t, oh, 64 * par : 64 * par + 65],
                        start=(oh == 0),
                        stop=(oh == 1),
                    )
            xcs = xs.tile([8, 8, 66], F32, tag="xcs")
            nc.vector.tensor_copy(xcs[:, 0:4, :], xca[:, :, 0:66])
            nc.vector.tensor_copy(xcs[:, 4:8, :], xcb[:, :, 0:66])
            nc.sync.dma_start(out=xcout_d[g], in_=xcs)

        def body(_i=None):
            st = emit_s1(0)
            for g in range(G):
                nb_sb, s2r = st
                v_sb, exp_sb = emit_s2(nb_sb, s2r)
                if g + 1 < G:
                    st = emit_s1(g + 1)
                emit_s4(g, v_sb, exp_sb)

        if repeat > 1:
            with tc.For_i(0, repeat, 1):
                body()
        else:
            body()

    nc.compile()
    _BUILD_CACHE[key] = nc
    return nc


def host_prep(pcd, neighbors, W_two, b_two, Wq, Wk, Wv):
    """Per-core input maps (host-side layout transforms + q/qc fold)."""
    scale = 1.0 / math.sqrt(D)
    # q[b,n,e] then qc[b,h,n,c] = sum_d q[b,n,(h,d)] Wk[(h,d),c] * scale
    q = np.einsum("bnc,ec->bne", pcd, Wq).astype(np.float32)
    qc = np.einsum("bnhd,hdc->bhnc", q.reshape(B, N, H, D), Wk.reshape(H, D, C))
    qc = (qc * scale).astype(np.float32)

    # w2t[p, a, o] with f = 128a + p
    w2t = np.ascontiguousarray(
        np.transpose(W_two.T.reshape(4, 128, O), (1, 0, 2))
    ).astype(ml_dtypes.bfloat16)
    b2 = b_two.reshape(1, O).astype(ml_dtypes.bfloat16)
    with_bias = bool(np.any(b_two))
    WvT = np.asarray(Wv).T.astype(np.float32)  # [c, e]

    in_maps = []
    npb = N // (NCORES // B)  # points per core
    for core in range(NCORES):
        b = core // (NCORES // B)
        n0 = (core % (NCORES // B)) * npb
        nb = np.asarray(neighbors[b, :, n0 : n0 + npb, :])  # (c, np, f)
        # nbt[g, p, a, cn]: f = 128a + p, cn = (n within group)*64 + c
        arr = np.transpose(nb, (2, 1, 0)).reshape(4, 128, G, 512)
        nbt = np.ascontiguousarray(np.transpose(arr, (2, 1, 0, 3))).astype(
            ml_dtypes.bfloat16
        )
        # s2r[g, cn, t, 144]: cols 0:128 blockdiag(WvT, WvT), 128:144 qc pair
        s2r = np.zeros((G, 128, 4, 144), np.float32)
        s2r[:, 0:64, :, 0:64] = WvT[None, :, None, :]
        s2r[:, 64:128, :, 64:128] = WvT[None, :, None, :]
        qq = qc[b, :, n0 : n0 + npb, :].reshape(H, G, 4, 2, C)  # [h,g,t,par,c]
        s2r[:, 0:64, :, 128:136] = np.transpose(qq[:, :, :, 0, :], (1, 3, 2, 0))
        s2r[:, 64:128, :, 136:144] = np.transpose(qq[:, :, :, 1, :], (1, 3, 2, 0))
        s2r = np.ascontiguousarray(s2r).astype(ml_dtypes.bfloat16)
        in_maps.append({"nbt": nbt, "w2t": w2t, "s2r": s2r, "b2": b2})
    return in_maps, with_bias


def kernel(pcd, neighbors, W_two, b_two, Wq, Wk, Wv):
    in_maps, with_bias = host_prep(pcd, neighbors, W_two, b_two, Wq, Wk, Wv)
    nc = build_nc(with_bias)
    res = run_bass_kernel_spmd(nc, in_maps, list(range(NCORES)))
    out = np.empty((B, C, N), np.float32)
    npb = N // (NCORES // B)
    hh = np.arange(C) // D  # head index per output channel
    ee = np.arange(C)
    for core in range(NCORES):
        b = core // (NCORES // B)
        n0 = (core % (NCORES // B)) * npb
        arr = res.results[core]["xcout"]                  # [G, h=8, nl=8, 66]
        bt = np.transpose(arr, (0, 2, 1, 3))              # [g, nl, h, col]
        xval = np.empty((G, 8, 8, 64), np.float32)
        zval = np.empty((G, 8, 8), np.float32)
        xval[:, 0::2] = bt[:, 0::2, :, 0:64]
        zval[:, 0::2] = bt[:, 0::2, :, 64]
        xval[:, 1::2] = bt[:, 1::2, :, 1:65]
        zval[:, 1::2] = bt[:, 1::2, :, 0]
        xc = xval[:, :, hh, ee]                           # [g, nl, C]
        zc = zval[:, :, hh]                               # [g, nl, C]
        o = xc / zc                                       # [g, nl, C]
        # n = 8g + nl
        out[b, :, n0 : n0 + npb] = np.transpose(o, (2, 0, 1)).reshape(C, NP)
    return out
